# revision 37
# baseline (speedup 1.0000x reference)
"""DFlash Qwen3 cross-attention on 8 TRN2 NeuronCores.

Sharding: tensor-parallel over heads. Core c owns KV head c (KVH=8) and the
4 query heads 4c..4c+3 of its GQA group.

v4 structure (evolved from baseline/v2/v3 trace analysis):
- All DMAs on HWDGE rings (sync = big streams + o_proj loads, scalar =
  cos/sin loads, AG staging, output stores).  No SWDGE descriptor-gen.
- Both projections are W-stationary: lhsT = weight chunk, moving = ckT
  columns at N=512, so Q^T and K^T come out of PSUM directly in the
  [d, pos] layout attention wants (no per-chunk PE transposes) and the
  KV matmul count halves vs the ck-stationary form.  V^T is transposed
  back to natural via 48 PE transposes.
- RMSNorm in transposed space: evac PSUM->bf16 SBUF, square (DVE 2x),
  partition-sum via a bf16 ones-matmul on the tensor engine (output is
  broadcast over partitions), ACT sqrt + reciprocal_approx_fast, rope
  via two host-precomputed transposed cos/sin tiles (norm weight folded
  in), final per-column 1/rms multiply writes qT/kT directly.
- Attention: j outermost (AG -> o_proj one j-tile behind), head pairs
  share kT/v stationaries, ONE [128,1024] exp per r-step, softmax
  denominator on vector only via bf16 pair-sums + f32 accumulate.
- Sweep end: partition-reduce of the denominator via an f32 ones-matmul
  (tensor), reciprocal_approx_fast, normalize straight out of PSUM.
  gpsimd runs ONLY the collective triggers: the AllGather trigger blocks
  its queue until the collective completes (~20us), which in v3 starved
  partition_all_reduce and stalled the whole pipeline at j boundaries.
"""

from contextlib import ExitStack

import numpy as np
from ml_dtypes import bfloat16

import concourse.bass as bass
import concourse.bass_isa as bass_isa
import concourse.mybir as mybir
import concourse.tile as tile
from concourse import bacc
from concourse.bass_utils import run_bass_kernel_spmd
from concourse.masks import make_identity

H = 32
KVH = 8
D = 128
HID = 4096
CTX = 4096
QL = 2048
KV = CTX + QL  # 6144
NCORES = 8
HPC = H // NCORES  # 4 q heads per core
THETA = 1000000.0
EPS = 1e-6
SCALE = float(D) ** -0.5

NHD = HID // 128  # 32 contraction chunks
NKV = KV // 128  # 48 kv chunks
NQC = QL // 128  # 16 q row chunks
NQJ = QL // 512  # 4 q column tiles for attention
MASKVAL = -1e6

F32 = mybir.dt.float32
BF16 = mybir.dt.bfloat16
MULT = mybir.AluOpType.mult

_STATE = {}


def _build():
    nc = bacc.Bacc()

    ckT = nc.declare_dram_parameter("ckT", [HID, KV], BF16, isOutput=False)
    wq = nc.declare_dram_parameter("wq", [HID, HPC * D], BF16, isOutput=False)
    wkv = nc.declare_dram_parameter("wkv", [HID, 2 * D], BF16, isOutput=False)
    wo = nc.declare_dram_parameter("wo", [HID, HPC * D], BF16, isOutput=False)
    csqT = nc.declare_dram_parameter("csqT", [2 * D, QL], BF16, isOutput=False)
    cskT = nc.declare_dram_parameter("cskT", [2 * D, KV], BF16, isOutput=False)
    msk = nc.declare_dram_parameter("msk", [128, 4 * 512], BF16, isOutput=False)
    rot = nc.declare_dram_parameter("rot", [128, 128], BF16, isOutput=False)
    out_ext = nc.declare_dram_parameter("out", [QL, HPC * D], F32, isOutput=True)

    ag_ins = [[nc.dram_tensor(f"ag_in{hp}_{j}", [128, 1024], BF16) for j in range(NQJ)]
              for hp in range(2)]
    ag_outs = [[nc.dram_tensor(f"ag_out{hp}_{j}", [NCORES * 128, 1024], BF16,
                               addr_space="Shared") for j in range(NQJ)]
               for hp in range(2)]

    with tile.TileContext(nc) as tc, ExitStack() as ctx:
        singles = ctx.enter_context(tc.tile_pool(name="singles", bufs=1))
        ck_pool = ctx.enter_context(tc.tile_pool(name="ckp", bufs=3))
        cs_pool = ctx.enter_context(tc.tile_pool(name="csp", bufs=2))
        evac_pool = ctx.enter_context(tc.tile_pool(name="evac", bufs=3))
        tmp_pool = ctx.enter_context(tc.tile_pool(name="tmp", bufs=2))
        pt_pool = ctx.enter_context(tc.tile_pool(name="ptp", bufs=4))
        sacc_pool = ctx.enter_context(tc.tile_pool(name="sacc", bufs=1))
        stg_pool = ctx.enter_context(tc.tile_pool(name="stg", bufs=2))
        at_pool = ctx.enter_context(tc.tile_pool(name="atp", bufs=2))
        # PSUM: A = proj accumulators / attention S^T (2 x 2 banks = 4);
        # B: "oacc" (2 banks) + "scr" (2 banks: proj norm-sums + V
        # transposes, attention denominator, o_proj accumulator)
        psumA = ctx.enter_context(tc.tile_pool(name="psA", bufs=2, space="PSUM"))
        psumB = ctx.enter_context(tc.tile_pool(name="psB", bufs=1, space="PSUM"))

        # ---- resident tensors ----
        wq_sb = singles.tile([128, NHD, HPC * D], BF16)
        for wpc in range(8):
            kk = NHD // 8
            nc.scalar.dma_start(
                out=wq_sb[:, wpc * kk:(wpc + 1) * kk, :],
                in_=wq[wpc * kk * 128:(wpc + 1) * kk * 128, :].rearrange(
                    "(k p) n -> p k n", p=128))
        wkv_sb = singles.tile([128, NHD, 2 * D], BF16)
        wo_sb = singles.tile([128, NHD, HPC * D], BF16)
        msk_sb = singles.tile([128, 4, 512], BF16)
        nc.scalar.dma_start(out=msk_sb[:], in_=msk[:, :].rearrange("p (i c) -> p i c", i=4))

        identb = singles.tile([128, 128], BF16)
        make_identity(nc, identb)
        rot_sb = singles.tile([128, 128], BF16)
        nc.scalar.dma_start(out=rot_sb, in_=rot[:, :])
        ones_b = singles.tile([128, 128], BF16)
        nc.vector.memset(ones_b, 1.0)
        ones_f = singles.tile([128, 128], F32)
        nc.vector.memset(ones_f, 1.0)
        epst = singles.tile([128, 1], F32)
        nc.vector.memset(epst, EPS)
        zbias = singles.tile([128, 1], F32)
        nc.vector.memset(zbias, 0.0)

        qT_sb = singles.tile([128, HPC, QL], BF16)  # Q^T per head: [d, h, q]
        kT_sb = singles.tile([128, KV], BF16)  # K^T: [d, kv]
        v_sb = singles.tile([128, NKV, D], BF16)  # V: [kv%128, r, d]

        pend1 = [None]
        pend2 = [None]

        def step_pipeline(new_stage1=None):
            if pend2[0] is not None:
                pend2[0]()
            pend2[0] = None
            if pend1[0] is not None:
                pend2[0] = pend1[0]()
            pend1[0] = new_stage1

        def norm_rope_T(xb, cst, out_slice):
            """Transposed-space rmsnorm+rope for one [128, 512] tile.

            xb: [128, 512] bf16 (rows = d, cols = positions).
            cst: [128, 2, 512] bf16 (A_T, B_T with norm weight folded in).
            The d -> (d+64)%128 partition rotation the rope needs is done
            with a permutation matmul (DVE lanes cannot shift partitions).
            Writes normalized+roped bf16 into out_slice ([128, 512])."""
            sq = tmp_pool.tile([128, 512], BF16, tag="sq")
            nc.vector.tensor_mul(sq, xb, xb)
            scr = psumB.tile([128, 2, 512], F32, tag="scr", name="normscr")
            nc.tensor.matmul(scr[:, 0, :], lhsT=ones_b, rhs=sq, start=True, stop=True)
            nc.tensor.matmul(scr[:, 1, :], lhsT=rot_sb, rhs=xb, start=True, stop=True)
            sqr = tmp_pool.tile([128, 512], F32, tag="sqr", bufs=1)
            nc.scalar.activation(out=sqr, in_=scr[:, 0, :],
                                 func=mybir.ActivationFunctionType.Sqrt,
                                 bias=epst, scale=1.0 / D)
            rs = tmp_pool.tile([128, 512], F32, tag="rs", bufs=1)
            nc.vector.reciprocal_approx_fast(out=rs, in_=sqr)
            t1 = tmp_pool.tile([128, 512], BF16, tag="t1")
            nc.vector.tensor_mul(t1, xb, cst[:, 0, :])
            t2 = tmp_pool.tile([128, 512], BF16, tag="t2")
            nc.vector.tensor_mul(t2, scr[:, 1, :], cst[:, 1, :])
            rsum = tmp_pool.tile([128, 512], BF16, tag="rsum")
            nc.vector.tensor_add(rsum, t1, t2)
            nc.vector.tensor_mul(out_slice, rsum, rs)

        # ================= Q projection =================
        # W-stationary: psum tile [:, h%2, :] = qT of head h for this
        # 512-column group of q positions.
        def q_stage1(g, pq01):
            qb = evac_pool.tile([128, 4, 512], BF16, tag="qb", bufs=2)
            nc.scalar.copy(out=qb[:, 0:2, :], in_=pq01[0])
            nc.scalar.copy(out=qb[:, 2:4, :], in_=pq01[1])
            cst = cs_pool.tile([128, 2, 512], BF16, tag="cs")
            nc.scalar.dma_start(out=cst, in_=csqT[:, g * 512:(g + 1) * 512].rearrange(
                "(two p) c -> p two c", p=128))
            for h in range(HPC):
                norm_rope_T(qb[:, h, :], cst,
                            qT_sb[:, h, g * 512:(g + 1) * 512])
            return None

        for g in range(4):
            pq01 = [psumA.tile([128, 2, 512], F32, tag="acc", name=f"pq{g}_{i}")
                    for i in range(2)]
            for k2 in range(NHD // 2):
                ckq = ck_pool.tile([128, 2, 512], BF16, tag="ck", bufs=6)
                nc.sync.dma_start(
                    out=ckq,
                    in_=ckT[k2 * 256:(k2 + 1) * 256,
                            CTX + g * 512: CTX + (g + 1) * 512].rearrange(
                        "(two p) c -> p two c", p=128))
                for two in range(2):
                    k = 2 * k2 + two
                    for h in range(HPC):
                        nc.tensor.matmul(pq01[h // 2][:, h % 2, :],
                                         lhsT=wq_sb[:, k, h * 128:(h + 1) * 128],
                                         rhs=ckq[:, two, :], start=(k == 0), stop=(k == NHD - 1))
            step_pipeline(lambda g=g, pq01=pq01: q_stage1(g, pq01))
            if g == 0:
                for wpc in range(2):
                    kk = NHD // 2
                    nc.gpsimd.dma_start(
                        out=wkv_sb[:, wpc * kk:(wpc + 1) * kk, :],
                        in_=wkv[wpc * kk * 128:(wpc + 1) * kk * 128, :].rearrange(
                            "(k p) n -> p k n", p=128))
            if g == 1:
                nc.gpsimd.dma_start(
                    out=wo_sb[:], in_=wo[:, :].rearrange("(k p) n -> p k n", p=128))

        # ================= K/V projection =================
        # W-stationary: pk[:, 0, :] = K^T, pk[:, 1, :] = V^T for this
        # 512-column group of kv positions.
        def kv_stage1(rq, pk):
            kb = evac_pool.tile([128, 2, 512], BF16, tag="kb")
            nc.scalar.copy(out=kb, in_=pk)
            cst = cs_pool.tile([128, 2, 512], BF16, tag="cs")
            nc.scalar.dma_start(out=cst, in_=cskT[:, rq * 512:(rq + 1) * 512].rearrange(
                "(two p) c -> p two c", p=128))
            norm_rope_T(kb[:, 0, :], cst, kT_sb[:, rq * 512:(rq + 1) * 512])

            def kv_stage2(rq=rq, kb=kb):
                tpv = psumB.tile([128, 512], BF16, tag="oacc", name="tpv")
                for rr in range(4):
                    nc.tensor.transpose(tpv[:, rr * 128:(rr + 1) * 128],
                                        kb[:, 1, rr * 128:(rr + 1) * 128], identb)
                nc.scalar.copy(out=v_sb[:, rq * 4:(rq + 1) * 4, :],
                               in_=tpv.rearrange("p (r d) -> p r d", r=4))
            return kv_stage2

        for rq in range(NKV // 4):
            pk = psumA.tile([128, 2, 512], F32, tag="acc", name=f"pk{rq}")
            for k2 in range(NHD // 2):
                ckt = ck_pool.tile([128, 2, 512], BF16, tag="ck", bufs=6)
                nc.sync.dma_start(
                    out=ckt,
                    in_=ckT[k2 * 256:(k2 + 1) * 256,
                            rq * 512:(rq + 1) * 512].rearrange("(two p) c -> p two c", p=128))
                for two in range(2):
                    k = 2 * k2 + two
                    for half in range(2):
                        nc.tensor.matmul(pk[:, half, :],
                                         lhsT=wkv_sb[:, k, half * 128:(half + 1) * 128],
                                         rhs=ckt[:, two, :], start=(k == 0), stop=(k == NHD - 1))
            step_pipeline(lambda rq=rq, pk=pk: kv_stage1(rq, pk))

        step_pipeline()
        step_pipeline()

        # ================= attention =================
        ones_rg = [list(range(NCORES))]

        def emit_oproj(jq, final=False):
            if not final:
                for qc in range(4 * jq, 4 * jq + 4):
                    qo = (qc % 4) * 128
                    po = psumB.tile([128, 512], F32, tag="oacc", name="po")
                    first = True
                    for hp in range(2):
                        for hl in range(2):
                            at = at_pool.tile([128, NCORES, 128], BF16, tag="at")
                            nc.sync.dma_start(
                                out=at,
                                in_=ag_outs[hp][jq][:, hl * 512 + qo: hl * 512 + qo + 128]
                                .rearrange("(c p) q -> p c q", p=128))
                            for ci in range(NCORES):
                                nc.tensor.matmul(po, lhsT=at[:, ci, :],
                                                 rhs=wo_sb[:, HPC * ci + 2 * hp + hl, :],
                                                 start=first,
                                                 stop=(hp == 1 and hl == 1 and ci == NCORES - 1))
                                first = False
                    ot = stg_pool.tile([128, 512], F32, tag="ot")
                    nc.vector.tensor_copy(out=ot, in_=po)
                    nc.scalar.dma_start(out=out_ext[qc * 128:(qc + 1) * 128, :], in_=ot)
            else:
                # final j-tile: 4 po accumulators in the freed attention PSUM
                # ring; hp0 (whose AllGather landed a sweep ago) fully
                # consumed before the hp1 matmuls that wait on the last AG.
                pof = [psumA.tile([128, 2, 512], F32, tag="acc", name=f"pofin{i}")
                       for i in range(2)]
                for hp in range(2):
                    for hl in range(2):
                        for qc4 in range(4):
                            qc = 4 * jq + qc4
                            qo = (qc % 4) * 128
                            at = at_pool.tile([128, NCORES, 128], BF16, tag="at")
                            nc.sync.dma_start(
                                out=at,
                                in_=ag_outs[hp][jq][:, hl * 512 + qo: hl * 512 + qo + 128]
                                .rearrange("(c p) q -> p c q", p=128))
                            for ci in range(NCORES):
                                nc.tensor.matmul(pof[qc4 // 2][:, qc4 % 2, :],
                                                 lhsT=at[:, ci, :],
                                                 rhs=wo_sb[:, HPC * ci + 2 * hp + hl, :],
                                                 start=(hp == 0 and hl == 0 and ci == 0),
                                                 stop=(hp == 1 and hl == 1 and ci == NCORES - 1))
                for qc4 in range(4):
                    qc = 4 * jq + qc4
                    ot = stg_pool.tile([128, 512], F32, tag="ot")
                    nc.vector.tensor_copy(out=ot, in_=pof[qc4 // 2][:, qc4 % 2, :])
                    nc.scalar.dma_start(out=out_ext[qc * 128:(qc + 1) * 128, :], in_=ot)

        pend_sweep = [None]
        for j in range(NQJ):
            rmax = 35 + 4 * j if j < NQJ - 1 else NKV - 1
            for hp in range(2):
                par = "oacc" if (2 * j + hp) % 2 == 0 else "scr"
                opar = "scr" if par == "oacc" else "oacc"
                sacc = sacc_pool.tile([128, 2, 512], F32, tag="sacc", bufs=2)
                nc.vector.memset(sacc, 0.0)
                o_acc = psumB.tile([128, 2, 512], F32, tag=par, name="o_acc")
                pt_pair = []
                pv_q = []
                for r in range(rmax + 1):
                    # 3-deep S^T ring: two psumA bufs plus the psumB parity
                    # tag not holding this sweep's o_acc (its previous
                    # occupant, last sweep's denominator, is read at r~1)
                    if r % 3 == 2:
                        st = psumB.tile([128, 2, 512], F32, tag=opar, name="st_b")
                    else:
                        st = psumA.tile([128, 2, 512], F32, tag="acc")
                    for hl in range(2):
                        nc.tensor.matmul(st[:, hl, :], lhsT=kT_sb[:, r * 128:(r + 1) * 128],
                                         rhs=qT_sb[:, 2 * hp + hl, j * 512:(j + 1) * 512],
                                         start=True, stop=True)
                    i = r - 32 - 4 * j
                    if i >= 0:
                        w = 128 * (i + 1)
                        for hl in range(2):
                            nc.vector.tensor_add(st[:, hl, 0:w], st[:, hl, 0:w],
                                                 msk_sb[:, i, 0:w])
                    pt = pt_pool.tile([128, 2, 512], BF16, tag="pt")
                    nc.scalar.activation(out=pt, in_=st,
                                         func=mybir.ActivationFunctionType.Exp,
                                         bias=zbias, scale=SCALE)
                    if r == 1 and pend_sweep[0] is not None:
                        pend_sweep[0]()
                        pend_sweep[0] = None
                    # PV emitted one r behind so it never waits on the exp
                    pv_q.append((r, pt))
                    if len(pv_q) == 2:
                        rr, ptp = pv_q.pop(0)
                        for hl in range(2):
                            nc.tensor.matmul(o_acc[:, hl, :], lhsT=v_sb[:, rr, :],
                                             rhs=ptp[:, hl, :],
                                             start=(rr == 0), stop=False)
                    pt_pair.append(pt)
                    if len(pt_pair) == 2:
                        u = tmp_pool.tile([128, 2, 512], BF16, tag="u", bufs=1)
                        nc.vector.tensor_add(u, pt_pair[0], pt_pair[1])
                        nc.vector.tensor_add(sacc, sacc, u)
                        pt_pair = []
                if pt_pair:
                    nc.vector.tensor_add(sacc, sacc, pt_pair[0])
                    pt_pair = []
                rr, ptp = pv_q.pop(0)
                for hl in range(2):
                    nc.tensor.matmul(o_acc[:, hl, :], lhsT=v_sb[:, rr, :],
                                     rhs=ptp[:, hl, :],
                                     start=(rr == 0), stop=True)
                # sweep end: free o_acc promptly with a scalar evac; the
                # rest of the chain (denominator reduce, recip, normalize,
                # stage + AllGather) is emitted after the NEXT sweep's first
                # r-step so the tensor FIFO never idles on it.
                oraw = sacc_pool.tile([128, 2, 512], F32, tag="oraw")
                nc.scalar.copy(out=oraw, in_=o_acc)

                def sweep_end(hp=hp, j=j, sacc=sacc, oraw=oraw, par=par):
                    saccb = stg_pool.tile([128, 2, 512], BF16, tag="saccb", bufs=1)
                    nc.vector.tensor_copy(out=saccb, in_=sacc)
                    dps = psumB.tile([128, 2, 512], F32, tag=par, name="dps")
                    for hl in range(2):
                        nc.tensor.matmul(dps[:, hl, :], lhsT=ones_b, rhs=saccb[:, hl, :],
                                         start=True, stop=True)
                    pri = sacc_pool.tile([128, 2, 512], F32, tag="pri")
                    nc.vector.reciprocal_approx_fast(out=pri, in_=dps)
                    stg = stg_pool.tile([128, 2, 512], BF16, tag="stg")
                    nc.vector.tensor_mul(stg, oraw, pri)
                    nc.scalar.dma_start(out=ag_ins[hp][j][:],
                                        in_=stg.rearrange("p a b -> p (a b)"))
                    nc.gpsimd.collective_compute(
                        "AllGather",
                        mybir.AluOpType.bypass,
                        ins=[ag_ins[hp][j][:]],
                        outs=[ag_outs[hp][j][:]],
                        replica_groups=ones_rg,
                    )
                pend_sweep[0] = sweep_end
            if j >= 1:
                if pend_sweep[0] is not None:
                    pend_sweep[0]()
                    pend_sweep[0] = None
                emit_oproj(j - 1)
        if pend_sweep[0] is not None:
            pend_sweep[0]()
            pend_sweep[0] = None
        emit_oproj(NQJ - 1, final=True)

    nc.compile()
    return nc


def _host_prep(context, query, w_qkv, w_o, q_norm_w, k_norm_w):
    context = np.asarray(context, dtype=np.float32)
    query = np.asarray(query, dtype=np.float32)
    w_qkv = np.asarray(w_qkv, dtype=np.float32)
    w_o = np.asarray(w_o, dtype=np.float32)
    q_norm_w = np.asarray(q_norm_w, dtype=np.float32)
    k_norm_w = np.asarray(k_norm_w, dtype=np.float32)

    ck = np.concatenate([context, query], axis=0)  # [KV, HID]
    ckT = np.ascontiguousarray(ck.T).astype(bfloat16)  # [HID, KV]

    wq = w_qkv[:, :H * D]
    wk = w_qkv[:, H * D:H * D + KVH * D]
    wv = w_qkv[:, H * D + KVH * D:]

    half = D // 2
    inv_freq = (1.0 / (THETA ** (np.arange(0, half, dtype=np.float32) / half))).astype(np.float32)
    pos = np.arange(KV, dtype=np.float32)
    freqs = pos[:, None] * inv_freq[None, :]   # [KV, 64]
    c = np.cos(freqs).T                        # [64, KV]
    s = np.sin(freqs).T

    def make_csT(nw):
        nw1 = nw[:half, None]
        nw2 = nw[half:, None]
        A = np.concatenate([c * nw1, c * nw2], axis=0)       # [128, KV]
        B = np.concatenate([-s * nw2, s * nw1], axis=0)      # [128, KV]
        return np.concatenate([A, B], axis=0).astype(bfloat16)  # [256, KV]

    cskT_full = make_csT(k_norm_w)
    csqT_full = make_csT(q_norm_w)[:, CTX:]

    p = np.arange(128)[:, None]
    q = np.arange(512)[None, :]
    msk = np.concatenate(
        [np.where(128 * i + p <= q, 0.0, MASKVAL) for i in range(4)],
        axis=1).astype(bfloat16)  # [128, 2048]

    rot = np.zeros((128, 128), dtype=np.float32)
    rot[(np.arange(128) + 64) % 128, np.arange(128)] = 1.0
    rot = rot.astype(bfloat16)

    in_maps = []
    for cidx in range(NCORES):
        in_maps.append({
            "ckT": ckT,
            "wq": np.ascontiguousarray(wq[:, cidx * HPC * D:(cidx + 1) * HPC * D]).astype(bfloat16),
            "wkv": np.ascontiguousarray(
                np.concatenate([wk[:, cidx * D:(cidx + 1) * D], wv[:, cidx * D:(cidx + 1) * D]], axis=1)
            ).astype(bfloat16),
            "wo": np.ascontiguousarray(w_o[:, cidx * HPC * D:(cidx + 1) * HPC * D]).astype(bfloat16),
            "csqT": csqT_full,
            "cskT": cskT_full,
            "msk": msk,
            "rot": rot,
        })
    return in_maps


def kernel(context, query, w_qkv, w_o, q_norm_w, k_norm_w, **kw):
    if "nc" not in _STATE:
        _STATE["nc"] = _build()
    nc = _STATE["nc"]
    in_maps = _host_prep(context, query, w_qkv, w_o, q_norm_w, k_norm_w)
    try:
        res = run_bass_kernel_spmd(nc, in_maps, list(range(NCORES)), **kw)
    except Exception:
        # transient NRT device wedges have been observed to clear on retry
        res = run_bass_kernel_spmd(nc, in_maps, list(range(NCORES)), **kw)
    out = np.concatenate([np.asarray(res.results[c]["out"]) for c in range(NCORES)], axis=1)
    if kw:
        return out.astype(np.float32), res
    return out.astype(np.float32)
